# revision 20
# baseline (speedup 1.0000x reference)
"""Trainium2 Bass kernel for nn_NetRGCN (RGCN conv -> two linear heads on node 0).

Key structure: the module's output is
    (log_softmax(W_global @ h0 + b_global), log_softmax(W_sense @ h0 + b_sense))
with h0 = relu(rgcn_conv(x, ...)[0]).  Under per-(target-node, relation) mean
aggregation, h0 depends only on edges whose TARGET is node 0 (~E/N of the 800k
edges) plus x[0] @ W_root.  The kernel:

  phase 1 (edges sharded 8 ways): each core scans its 100k-edge shard
    on-device, compacts the (src, type) pairs of edges targeting node 0 with
    gpsimd sparse_gather, indirect-DMA-gathers just those x rows, and reduces
    them into per-relation sums + counts with relation-onehot PE matmuls.
  AllReduce: [5, 257] per-relation sums+counts across the 8 cores.
  phase 2: every core computes mean -> per-relation transform + root term ->
    h0 (tiny, replicated), then its 5632-row slice of the stacked padded
    [45056, 256] head matrix as PE matvecs plus log-softmax partial stats.
  AllGather: 4 stats floats per core; each core normalizes its slice of the
    final log-probs on device.

Host work is limited to slicing/padding/transposing inputs and concatenating
the 8 output slices.
"""

import numpy as np

import concourse.bass as bass
import concourse.bacc as bacc
import concourse.mybir as mybir
import concourse.tile as tile
from concourse.bass_utils import run_bass_kernel_spmd

F32 = mybir.dt.float32
I32 = mybir.dt.int32
U32 = mybir.dt.uint32
AL = mybir.AluOpType
ACT = mybir.ActivationFunctionType
AX = mybir.AxisListType

# problem sizes (hardcoded per contract)
N, C, R = 50000, 256, 5
E = 800000
GLOBALS, SENSES = 25000, 20000
HEAD = GLOBALS + SENSES          # 45000
NCORES = 8
P = 128

ESH = E // NCORES                # 100000 edges per core
FREE = 784                       # 128*784 = 100352 padded shard
SH_PAD = P * FREE
KB = 8                           # top-8 matched edges per partition -> 8
                                 # gather batches of 128 (capacity 1024/core)

HEAD_PAD = 45056                 # 8 * 5632
HSH = HEAD_PAD // NCORES         # 5632 head rows per core
Q = HSH // P                     # 44 column blocks of 128 in the 2D layout
CH = C // P                      # 2 contraction chunks of 128

NEG = -1.0e30


def build_program() -> bass.Bass:
    # Bacc (not raw Bass): its compile() pass legalizes multi-wait sync via
    # event semaphores, auto-inserts gpsimd library loads for sparse_gather,
    # and encodes extended-ISA instruction bytes -- all required by walrus.
    nc = bacc.Bacc("TRN2", target_bir_lowering=False, debug=False,
                   num_devices=NCORES)

    # ---- per-core I/O -------------------------------------------------
    x_d = nc.declare_dram_parameter("x", [N, C], F32, isOutput=False)
    src_d = nc.declare_dram_parameter("src", [P, FREE], I32, isOutput=False)
    tgt_d = nc.declare_dram_parameter("tgt", [P, FREE], I32, isOutput=False)
    typ_d = nc.declare_dram_parameter("typ", [P, FREE], I32, isOutput=False)
    waug_d = nc.declare_dram_parameter("waug", [(R + 1) * C, C], F32, isOutput=False)
    bconv_d = nc.declare_dram_parameter("bconv", [1, C], F32, isOutput=False)
    whT_d = nc.declare_dram_parameter("whT", [C, HSH], F32, isOutput=False)
    bh_d = nc.declare_dram_parameter("bh", [P, Q], F32, isOutput=False)
    mg_d = nc.declare_dram_parameter("mg", [P, Q], F32, isOutput=False)
    ms_d = nc.declare_dram_parameter("ms", [P, Q], F32, isOutput=False)
    mgneg_d = nc.declare_dram_parameter("mgneg", [P, Q], F32, isOutput=False)
    msneg_d = nc.declare_dram_parameter("msneg", [P, Q], F32, isOutput=False)

    out_d = nc.declare_dram_parameter("out", [P, Q], F32, isOutput=True)
    dbg_d = nc.declare_dram_parameter("dbg", [1, 8], F32, isOutput=True)

    rg = [list(range(NCORES))]

    # collectives need whole internal DRAM tensors -- DRAM *pool tiles* are
    # offset views into an arena and break collectives on HW (sim passes).
    cc_in = nc.dram_tensor("cc_in", [NCORES, C + 8], F32)
    cc_out = nc.dram_tensor("cc_out", [NCORES, C + 8], F32)
    st_in = nc.dram_tensor("st_in", [1, 4], F32)
    st_out = nc.dram_tensor("st_out", [NCORES, 4], F32)

    with tile.TileContext(nc) as tc:
        with (
            tc.tile_pool(name="const", bufs=1) as cpool,
            tc.tile_pool(name="sb", bufs=1) as sb,
            tc.tile_pool(name="ps", bufs=1, space="PSUM") as ps,
            tc.tile_pool(name="pst", bufs=2, space="PSUM") as pst,
        ):
            # ---- constants (inline Const tensors; avoids gpsimd
            #      standard-library ops that would fight the sparse_gather
            #      library reload for the single Q7 library slot) ---------
            ident_c = nc.inline_tensor(np.eye(P, dtype=np.float32), name="ident_c")
            ident = cpool.tile([P, P], F32, tag="ident")
            nc.sync.dma_start(ident[:], ident_c[:])
            iota_c = nc.inline_tensor(
                np.tile(np.arange(R, dtype=np.float32), (P, 1)), name="iota_c")
            iotaf = cpool.tile([P, R], F32, tag="iotaf")
            nc.sync.dma_start(iotaf[:], iota_c[:])

            # ---- big weight loads (kick off early, overlap all) ------
            wtall = sb.tile([P, CH, HSH], F32, tag="wtall")
            nc.sync.dma_start(wtall[:], whT_d[:].rearrange("(h p) j -> p h j", p=P))
            wa = sb.tile([P, (R + 1) * CH, C], F32, tag="wa")
            nc.sync.dma_start(wa[:], waug_d[:].rearrange("(k p) d -> p k d", p=P))
            bh2 = sb.tile([P, Q], F32, tag="bh2")
            nc.sync.dma_start(bh2[:], bh_d[:])
            mg2 = sb.tile([P, Q], F32, tag="mg2")
            nc.sync.dma_start(mg2[:], mg_d[:])
            ms2 = sb.tile([P, Q], F32, tag="ms2")
            nc.sync.dma_start(ms2[:], ms_d[:])
            mgneg2 = sb.tile([P, Q], F32, tag="mgneg2")
            nc.sync.dma_start(mgneg2[:], mgneg_d[:])
            msneg2 = sb.tile([P, Q], F32, tag="msneg2")
            nc.sync.dma_start(msneg2[:], msneg_d[:])
            bconv_t = sb.tile([1, C], F32, tag="bconv_t")
            nc.sync.dma_start(bconv_t[:], bconv_d[:])

            # ---- phase 1: edge scan ----------------------------------
            src_i = sb.tile([P, FREE], I32, tag="src_i")
            nc.sync.dma_start(src_i[:], src_d[:])
            tgt_i = sb.tile([P, FREE], I32, tag="tgt_i")
            nc.sync.dma_start(tgt_i[:], tgt_d[:])
            typ_i = sb.tile([P, FREE], I32, tag="typ_i")
            nc.sync.dma_start(typ_i[:], typ_d[:])

            srcf = sb.tile([P, FREE], F32, tag="srcf")
            nc.vector.tensor_copy(srcf[:], src_i[:])
            tgtf = sb.tile([P, FREE], F32, tag="tgtf")
            nc.vector.tensor_copy(tgtf[:], tgt_i[:])
            typf = sb.tile([P, FREE], F32, tag="typf")
            nc.vector.tensor_copy(typf[:], typ_i[:])

            mask = sb.tile([P, FREE], F32, tag="mask")
            nc.vector.tensor_scalar(
                out=mask[:], in0=tgtf[:], scalar1=0.0, scalar2=None,
                op0=AL.is_equal,
            )
            # enc = (src*8 + type + 1) * mask - 1  -> combined id if the edge
            # targets node 0, else -1.  Exact in f32 (max 400001 < 2^24).
            enc = sb.tile([P, FREE], F32, tag="enc")
            nc.vector.scalar_tensor_tensor(
                out=enc[:], in0=srcf[:], scalar=8.0, in1=typf[:],
                op0=AL.mult, op1=AL.add,
            )
            # t = src*8+typ; matched -> t+1 (>0 even for src=typ=0), else -1:
            # (t+2)*mask - 1
            nc.vector.tensor_scalar_add(enc[:], enc[:], 2.0)
            nc.vector.tensor_tensor(out=enc[:], in0=enc[:], in1=mask[:],
                                    op=AL.mult)
            nc.vector.tensor_scalar_add(enc[:], enc[:], -1.0)

            # per-partition top-8 (native DVE InstMax; the Q7 sparse_gather
            # library op crashes this runtime).  Matched encodes are > 0 and
            # rare (~2 per core over 1024 slots); -1 fills the rest.
            topv = sb.tile([P, KB], F32, tag="topv")
            nc.vector.max(topv[:], enc[:])

            # decode: valid flag, relation, source row
            valid = sb.tile([P, KB], F32, tag="valid")
            nc.vector.tensor_scalar(
                out=valid[:], in0=topv[:], scalar1=0.0, scalar2=None,
                op0=AL.is_gt,
            )
            dec = sb.tile([P, KB], F32, tag="dec")
            nc.vector.tensor_scalar(
                out=dec[:], in0=topv[:], scalar1=1.0, scalar2=-1.0,
                op0=AL.max, op1=AL.add,
            )
            # dec = src*8 + typ (exact in f32).  DVE has no mod/floor, so
            # split via int cast with a +-8 correction that is exact under
            # any cast rounding mode.
            u = sb.tile([P, KB], F32, tag="u")
            nc.vector.tensor_scalar_mul(u[:], dec[:], 0.125)
            srcA = sb.tile([P, KB], I32, tag="srcA")
            nc.vector.tensor_copy(srcA[:], u[:])
            srcAf = sb.tile([P, KB], F32, tag="srcAf")
            nc.vector.tensor_copy(srcAf[:], srcA[:])
            err = sb.tile([P, KB], F32, tag="err")
            nc.vector.scalar_tensor_tensor(
                out=err[:], in0=srcAf[:], scalar=-8.0, in1=dec[:],
                op0=AL.mult, op1=AL.add,
            )  # err = dec - 8*srcA  in {typ-8, typ, typ+8}
            adj = sb.tile([P, KB], F32, tag="adj")
            nc.vector.tensor_scalar(
                out=adj[:], in0=err[:], scalar1=0.0, scalar2=8.0,
                op0=AL.is_lt, op1=AL.mult,
            )  # 8 where err<0
            typ8 = sb.tile([P, KB], F32, tag="typ8")
            nc.vector.tensor_tensor(out=typ8[:], in0=err[:], in1=adj[:],
                                    op=AL.add)
            nc.vector.tensor_scalar(
                out=adj[:], in0=typ8[:], scalar1=8.0, scalar2=8.0,
                op0=AL.is_ge, op1=AL.mult,
            )  # 8 where typ8>=8
            nc.vector.tensor_tensor(out=typ8[:], in0=typ8[:], in1=adj[:],
                                    op=AL.subtract)
            src8 = sb.tile([P, KB], F32, tag="src8")
            nc.vector.tensor_tensor(out=src8[:], in0=dec[:], in1=typ8[:],
                                    op=AL.subtract)
            nc.vector.tensor_scalar_mul(src8[:], src8[:], 0.125)
            idx = sb.tile([P, KB], I32, tag="idx")
            idx_cast = nc.vector.tensor_copy(idx[:], src8[:])

            # gather matched x rows (row 0 for empty slots; weight 0).
            # an extra column of ones makes one matmul yield counts too.
            psum_sum = ps.tile([R, C + 1], F32, tag="acc")
            for g in range(KB):
                xg = sb.tile([P, C + 1], F32, tag=f"xg{g}")
                gi = nc.gpsimd.indirect_dma_start(
                    out=xg[:, 0:C],
                    out_offset=None,
                    in_=x_d[:],
                    in_offset=bass.IndirectOffsetOnAxis(ap=idx[:, g:g + 1], axis=0),
                )
                # Tile does not track the offset-AP dependency of indirect
                # DMAs; without this edge the gather races the idx cast.
                tile.add_dep_helper(gi.ins, idx_cast.ins, True,
                                    "gather waits for idx cast")
                nc.vector.memset(xg[:, C:C + 1], 1.0)
                sg = sb.tile([P, R], F32, tag=f"sg{g}")
                nc.vector.tensor_tensor(
                    out=sg[:],
                    in0=typ8[:, g:g + 1].to_broadcast([P, R]),
                    in1=iotaf[:],
                    op=AL.is_equal,
                )
                nc.vector.tensor_tensor(
                    out=sg[:],
                    in0=valid[:, g:g + 1].to_broadcast([P, R]),
                    in1=sg[:],
                    op=AL.mult,
                )
                nc.tensor.matmul(
                    out=psum_sum[:], lhsT=sg[:], rhs=xg[:],
                    start=(g == 0), stop=(g == KB - 1),
                )

            pack = sb.tile([NCORES, C + 8], F32, tag="pack")
            nc.vector.memset(pack[:], 0.0)
            nc.vector.tensor_copy(pack[0:R, 0:C + 1], psum_sum[:])

            # ---- AllReduce of per-relation sums + counts -------------
            nc.sync.dma_start(cc_in[:], pack[:])
            nc.gpsimd.collective_compute(
                "AllReduce", AL.add, replica_groups=rg,
                ins=[cc_in[:]], outs=[cc_out[:]],
            )
            red = sb.tile([NCORES, C + 8], F32, tag="red")
            nc.sync.dma_start(red[:], cc_out[:])

            # ---- mean + RGCN transform for node 0 --------------------
            cnt_c = sb.tile([R, 1], F32, tag="cnt_c")
            nc.vector.tensor_scalar_max(cnt_c[:], red[0:R, C:C + 1], 1.0)
            rcp = sb.tile([R, 1], F32, tag="rcp")
            nc.vector.reciprocal(rcp[:], cnt_c[:])

            mean_aug = sb.tile([R + 1, C], F32, tag="mean_aug")
            nc.vector.tensor_scalar(
                out=mean_aug[0:R, :], in0=red[0:R, 0:C],
                scalar1=rcp[:], scalar2=None, op0=AL.mult,
            )
            nc.sync.dma_start(mean_aug[R:R + 1, :], x_d[0:1, :])

            # transpose -> [128, 6] x 2 chunks (contraction onto partitions)
            mts = []
            for h in range(CH):
                tp = pst.tile([P, R + 1], F32, tag="tr")
                nc.tensor.transpose(
                    out=tp[:],
                    in_=mean_aug[:, h * P:(h + 1) * P],
                    identity=ident[0:R + 1, 0:R + 1],
                )
                mt = sb.tile([P, R + 1], F32, tag=f"mt{h}")
                nc.vector.tensor_copy(mt[:], tp[:])
                mts.append(mt)

            # h0 = relu(sum_r mean_r @ W_rel[r] + x0 @ W_root + b_conv)
            h0ps = ps.tile([1, C], F32, tag="acc")
            nk = (R + 1) * CH
            for k in range(nk):
                nc.tensor.matmul(
                    out=h0ps[:],
                    lhsT=mts[k % CH][:, k // CH:k // CH + 1],
                    rhs=wa[:, k, :],
                    start=(k == 0), stop=(k == nk - 1),
                )
            h0 = sb.tile([1, C], F32, tag="h0")
            nc.vector.tensor_tensor(
                out=h0[:], in0=h0ps[:], in1=bconv_t[:], op=AL.add
            )
            nc.vector.tensor_scalar_max(h0[:], h0[:], 0.0)

            h0ts = []
            for h in range(CH):
                tp2 = pst.tile([P, 1], F32, tag="tr")
                nc.tensor.transpose(
                    out=tp2[:],
                    in_=h0[0:1, h * P:(h + 1) * P],
                    identity=ident[0:1, 0:1],
                )
                h0t = sb.tile([P, 1], F32, tag=f"h0t{h}")
                nc.vector.tensor_copy(h0t[:], tp2[:])
                h0ts.append(h0t)

            # ---- heads: logits2d[p, q] = logit of head row j = 128*q + p
            lg2 = ps.tile([P, Q], F32, tag="acc")
            for q in range(Q):
                for h in range(CH):
                    nc.tensor.matmul(
                        out=lg2[:, q:q + 1],
                        lhsT=wtall[:, h, q * P:(q + 1) * P],
                        rhs=h0ts[h][:],
                        start=(h == 0), stop=(h == CH - 1),
                    )
            logits2 = sb.tile([P, Q], F32, tag="logits2")
            nc.vector.tensor_tensor(
                out=logits2[:], in0=lg2[:], in1=bh2[:], op=AL.add
            )

            # ---- per-core log-softmax partials (flash style) ---------
            stats = sb.tile([1, 4], F32, tag="stats")
            for hi, (m2, mn2) in enumerate(((mg2, mgneg2), (ms2, msneg2))):
                masked = sb.tile([P, Q], F32, tag=f"masked{hi}")
                nc.vector.tensor_tensor(
                    out=masked[:], in0=logits2[:], in1=m2[:], op=AL.mult
                )
                nc.vector.tensor_tensor(
                    out=masked[:], in0=masked[:], in1=mn2[:], op=AL.add
                )
                pmax = sb.tile([P, 1], F32, tag=f"pmax{hi}")
                nc.vector.reduce_max(pmax[:], masked[:], axis=AX.X)
                npmax = sb.tile([P, 1], F32, tag=f"npmax{hi}")
                nc.vector.tensor_scalar_mul(npmax[:], pmax[:], -1.0)
                exps = sb.tile([P, Q], F32, tag=f"exps{hi}")
                esum = sb.tile([P, 1], F32, tag=f"esum{hi}")
                nc.scalar.activation(
                    out=exps[:], in_=masked[:], func=ACT.Exp,
                    bias=npmax[:], scale=1.0, accum_out=esum[:],
                )
                # fold 128 per-partition (m, s) pairs down to one (M, S)
                tpm = pst.tile([1, P], F32, tag="trv")
                nc.tensor.transpose(out=tpm[:], in_=pmax[:], identity=ident[:])
                mrow = sb.tile([1, P], F32, tag=f"mrow{hi}")
                nc.vector.tensor_copy(mrow[:], tpm[:])
                tps = pst.tile([1, P], F32, tag="trv")
                nc.tensor.transpose(out=tps[:], in_=esum[:], identity=ident[:])
                srow = sb.tile([1, P], F32, tag=f"srow{hi}")
                nc.vector.tensor_copy(srow[:], tps[:])

                Mh = sb.tile([1, 1], F32, tag=f"Mh{hi}")
                nc.vector.reduce_max(Mh[:], mrow[:], axis=AX.X)
                nMh = sb.tile([1, 1], F32, tag=f"nMh{hi}")
                nc.vector.tensor_scalar_mul(nMh[:], Mh[:], -1.0)
                wex = sb.tile([1, P], F32, tag=f"wex{hi}")
                nc.scalar.activation(
                    out=wex[:], in_=mrow[:], func=ACT.Exp, bias=nMh[:], scale=1.0
                )
                sw = sb.tile([1, P], F32, tag=f"sw{hi}")
                nc.vector.tensor_tensor(out=sw[:], in0=wex[:], in1=srow[:], op=AL.mult)
                Sh = sb.tile([1, 1], F32, tag=f"Sh{hi}")
                nc.vector.reduce_sum(Sh[:], sw[:], axis=AX.X)

                nc.vector.tensor_copy(stats[0:1, 2 * hi:2 * hi + 1], Mh[:])
                nc.vector.tensor_copy(stats[0:1, 2 * hi + 1:2 * hi + 2], Sh[:])

            # ---- AllGather stats, final LSE per head -----------------
            nc.sync.dma_start(st_in[:], stats[:])
            nc.gpsimd.collective_compute(
                "AllGather", AL.bypass, replica_groups=rg,
                ins=[st_in[:]], outs=[st_out[:]],
            )
            stb = sb.tile([NCORES, 4], F32, tag="stb")
            nc.sync.dma_start(stb[:], st_out[:])
            statsf = sb.tile([1, 4 * NCORES], F32, tag="statsf")
            nc.gpsimd.dma_start(statsf[:], stb[:])
            # statsf[0, 4i + c]: core i, c in (Mg, Sg, Ms, Ss)

            lse_pair = sb.tile([1, 2], F32, tag="lse_pair")
            dbg_t = sb.tile([1, 8], F32, tag="dbg_t")
            nc.vector.memset(dbg_t[:], 0.0)
            for hi in range(2):
                mvals = statsf[0:1, 2 * hi::4]       # [1, 8]
                svals = statsf[0:1, 2 * hi + 1::4]   # [1, 8]
                MG = sb.tile([1, 1], F32, tag=f"MG{hi}")
                nc.vector.reduce_max(MG[:], mvals, axis=AX.X)
                nMG = sb.tile([1, 1], F32, tag=f"nMG{hi}")
                nc.vector.tensor_scalar_mul(nMG[:], MG[:], -1.0)
                wex2 = sb.tile([1, NCORES], F32, tag=f"wex2{hi}")
                nc.scalar.activation(
                    out=wex2[:], in_=mvals, func=ACT.Exp, bias=nMG[:], scale=1.0
                )
                sw2 = sb.tile([1, NCORES], F32, tag=f"sw2{hi}")
                nc.vector.tensor_tensor(out=sw2[:], in0=wex2[:], in1=svals, op=AL.mult)
                SG = sb.tile([1, 1], F32, tag=f"SG{hi}")
                nc.vector.reduce_sum(SG[:], sw2[:], axis=AX.X)
                lnS = sb.tile([1, 1], F32, tag=f"lnS{hi}")
                nc.scalar.activation(
                    out=lnS[:], in_=SG[:], func=ACT.Ln, bias=0.0, scale=1.0
                )
                nc.vector.tensor_tensor(
                    out=lse_pair[0:1, hi:hi + 1], in0=MG[:], in1=lnS[:], op=AL.add
                )
                nc.vector.tensor_copy(dbg_t[0:1, 2 + hi:3 + hi], MG[:])

            # broadcast the two LSE scalars to all 128 partitions via PE
            ones_row = cpool.tile([1, P], F32, tag="ones_row")
            nc.vector.memset(ones_row[:], 1.0)
            bc_ps = pst.tile([P, 2], F32, tag="tr")
            nc.tensor.matmul(
                out=bc_ps[:], lhsT=ones_row[:], rhs=lse_pair[:],
                start=True, stop=True,
            )
            lseb = sb.tile([P, 2], F32, tag="lseb")
            nc.vector.tensor_copy(lseb[:], bc_ps[:])

            # out = logits - mg*LSE_g - ms*LSE_s
            t1 = sb.tile([P, Q], F32, tag="t1")
            nc.vector.tensor_scalar(
                out=t1[:], in0=mg2[:], scalar1=lseb[:, 0:1], scalar2=None,
                op0=AL.mult,
            )
            t2 = sb.tile([P, Q], F32, tag="t2")
            nc.vector.tensor_scalar(
                out=t2[:], in0=ms2[:], scalar1=lseb[:, 1:2], scalar2=None,
                op0=AL.mult,
            )
            nc.vector.tensor_tensor(out=t1[:], in0=t1[:], in1=t2[:], op=AL.add)
            out2 = sb.tile([P, Q], F32, tag="out2")
            nc.vector.tensor_tensor(
                out=out2[:], in0=logits2[:], in1=t1[:], op=AL.subtract
            )
            nc.sync.dma_start(out_d[:], out2[:])

            # debug output: total matched-edge count for this core
            nvalid = sb.tile([P, 1], F32, tag="nvalid")
            nc.vector.reduce_sum(nvalid[:], valid[:], axis=AX.X)
            tnv = pst.tile([1, P], F32, tag="trv")
            nc.tensor.transpose(out=tnv[:], in_=nvalid[:], identity=ident[:])
            nvrow = sb.tile([1, P], F32, tag="nvrow")
            nc.vector.tensor_copy(nvrow[:], tnv[:])
            nc.vector.reduce_sum(dbg_t[0:1, 0:1], nvrow[:], axis=AX.X)
            nc.sync.dma_start(dbg_d[:], dbg_t[:])

    nc.compile()
    return nc


def _shuffle_head_vec(vec: np.ndarray) -> np.ndarray:
    """[HSH] -> [P, Q] with row j = 128*q + p at [p, q]."""
    return np.ascontiguousarray(vec.reshape(Q, P).T)


def make_in_maps(x, edge_index, edge_type, W_rel, W_root, b_conv,
                 W_global, b_global, W_sense, b_sense):
    x = np.ascontiguousarray(np.asarray(x, dtype=np.float32))
    src_all = np.asarray(edge_index[0], dtype=np.int32)
    tgt_all = np.asarray(edge_index[1], dtype=np.int32)
    typ_all = np.asarray(edge_type, dtype=np.int32)

    waug = np.ascontiguousarray(np.concatenate(
        [np.asarray(W_rel, dtype=np.float32).reshape(R * C, C),
         np.asarray(W_root, dtype=np.float32)], axis=0))
    bconv = np.ascontiguousarray(
        np.asarray(b_conv, dtype=np.float32).reshape(1, C))

    w_head = np.concatenate(
        [np.asarray(W_global, dtype=np.float32),
         np.asarray(W_sense, dtype=np.float32),
         np.zeros((HEAD_PAD - HEAD, C), dtype=np.float32)], axis=0)
    w_head_T = np.ascontiguousarray(w_head.T)          # [C, HEAD_PAD]
    bh_vec = np.concatenate(
        [np.asarray(b_global, dtype=np.float32),
         np.asarray(b_sense, dtype=np.float32),
         np.zeros(HEAD_PAD - HEAD, dtype=np.float32)])
    jab = np.arange(HEAD_PAD)
    mg_vec = (jab < GLOBALS).astype(np.float32)
    ms_vec = ((jab >= GLOBALS) & (jab < HEAD)).astype(np.float32)
    mgneg_vec = (1.0 - mg_vec) * NEG
    msneg_vec = (1.0 - ms_vec) * NEG

    in_maps = []
    for i in range(NCORES):
        sl = slice(i * ESH, (i + 1) * ESH)
        pad = SH_PAD - ESH
        src_i = np.concatenate([src_all[sl], np.zeros(pad, np.int32)])
        tgt_i = np.concatenate([tgt_all[sl], np.ones(pad, np.int32)])
        typ_i = np.concatenate([typ_all[sl], np.zeros(pad, np.int32)])
        hs = slice(i * HSH, (i + 1) * HSH)
        in_maps.append({
            "x": x,
            "src": np.ascontiguousarray(src_i.reshape(P, FREE)),
            "tgt": np.ascontiguousarray(tgt_i.reshape(P, FREE)),
            "typ": np.ascontiguousarray(typ_i.reshape(P, FREE)),
            "waug": waug,
            "bconv": bconv,
            "whT": np.ascontiguousarray(w_head_T[:, hs]),
            "bh": _shuffle_head_vec(bh_vec[hs]),
            "mg": _shuffle_head_vec(mg_vec[hs]),
            "ms": _shuffle_head_vec(ms_vec[hs]),
            "mgneg": _shuffle_head_vec(mgneg_vec[hs]),
            "msneg": _shuffle_head_vec(msneg_vec[hs]),
        })
    return in_maps


def postprocess(outs: list) -> tuple:
    flat = np.concatenate([np.asarray(o).T.reshape(-1) for o in outs])
    return (np.ascontiguousarray(flat[:GLOBALS], dtype=np.float32),
            np.ascontiguousarray(flat[GLOBALS:HEAD], dtype=np.float32))


_NC_CACHE = None


def kernel(**inputs) -> tuple:
    global _NC_CACHE
    if _NC_CACHE is None:
        _NC_CACHE = build_program()
    nc = _NC_CACHE
    in_maps = make_in_maps(**inputs)
    res = run_bass_kernel_spmd(nc, in_maps, list(range(NCORES)))
    return postprocess([r["out"] for r in res.results])


# revision 33
# speedup vs baseline: 1.1757x; 1.1757x over previous
"""Trainium2 Bass kernel for nn_NetRGCN (RGCN conv -> two linear heads on node 0).

Key structure: the module's output is
    (log_softmax(W_global @ h0 + b_global), log_softmax(W_sense @ h0 + b_sense))
with h0 = relu(rgcn_conv(x, ...)[0]).  Under per-(target-node, relation) mean
aggregation, h0 depends only on edges whose TARGET is node 0 (~E/N of the 800k
edges) plus x[0] @ W_root.  The kernel:

  phase 1 (edges sharded 8 ways): each core scans its 100k-edge shard
    on-device, compacts the (src, type) pairs of edges targeting node 0 with
    gpsimd sparse_gather, indirect-DMA-gathers just those x rows, and reduces
    them into per-relation sums + counts with relation-onehot PE matmuls.
  AllReduce: [5, 257] per-relation sums+counts across the 8 cores.
  phase 2: every core computes mean -> per-relation transform + root term ->
    h0 (tiny, replicated), then its 5632-row slice of the stacked padded
    [45056, 256] head matrix as PE matvecs plus log-softmax partial stats.
  AllGather: 4 stats floats per core; each core normalizes its slice of the
    final log-probs on device.

Host work is limited to slicing/padding/transposing inputs and concatenating
the 8 output slices.
"""

import numpy as np

import concourse.bass as bass
import concourse.bacc as bacc
import concourse.mybir as mybir
import concourse.tile as tile
from concourse.bass_utils import run_bass_kernel_spmd

F32 = mybir.dt.float32
I32 = mybir.dt.int32
U32 = mybir.dt.uint32
AL = mybir.AluOpType
ACT = mybir.ActivationFunctionType
AX = mybir.AxisListType

# problem sizes (hardcoded per contract)
N, C, R = 50000, 256, 5
E = 800000
GLOBALS, SENSES = 25000, 20000
HEAD = GLOBALS + SENSES          # 45000
NCORES = 8
P = 128

ESH = E // NCORES                # 100000 edges per core
FREE = 784                       # 128*784 = 100352 padded shard
SH_PAD = P * FREE
KB = 4                           # matched edges kept per partition (of the
                                 # DVE top-8) -> 4 gather batches of 128

HEAD_PAD = 45056                 # 8 * 5632
HSH = HEAD_PAD // NCORES         # 5632 head rows per core
Q = HSH // P                     # 44 columns in the [128, 44] 2D layout
JB = HSH // 512                  # 11 psum blocks of 512 head rows
CH = C // P                      # 2 contraction chunks of 128

NEG = -1.0e30


def build_program() -> bass.Bass:
    # Bacc (not raw Bass): its compile() pass legalizes multi-wait sync via
    # event semaphores, auto-inserts gpsimd library loads for sparse_gather,
    # and encodes extended-ISA instruction bytes -- all required by walrus.
    nc = bacc.Bacc("TRN2", target_bir_lowering=False, debug=False,
                   num_devices=NCORES)

    # ---- per-core I/O -------------------------------------------------
    x_d = nc.declare_dram_parameter("x", [N, C], F32, isOutput=False)
    src_d = nc.declare_dram_parameter("src", [P, FREE], I32, isOutput=False)
    tgt_d = nc.declare_dram_parameter("tgt", [P, FREE], I32, isOutput=False)
    typ_d = nc.declare_dram_parameter("typ", [P, FREE], I32, isOutput=False)
    waug_d = nc.declare_dram_parameter("waug", [(R + 1) * C, C], F32, isOutput=False)
    bconv_d = nc.declare_dram_parameter("bconv", [1, C], F32, isOutput=False)
    whT_d = nc.declare_dram_parameter("whT", [C, HSH], F32, isOutput=False)
    bh_d = nc.declare_dram_parameter("bh", [P, Q], F32, isOutput=False)
    mg_d = nc.declare_dram_parameter("mg", [P, Q], F32, isOutput=False)
    ms_d = nc.declare_dram_parameter("ms", [P, Q], F32, isOutput=False)
    mgneg_d = nc.declare_dram_parameter("mgneg", [P, Q], F32, isOutput=False)
    msneg_d = nc.declare_dram_parameter("msneg", [P, Q], F32, isOutput=False)

    out_d = nc.declare_dram_parameter("out", [P, Q], F32, isOutput=True)
    dbg_d = nc.declare_dram_parameter("dbg", [1, 8], F32, isOutput=True)

    rg = [list(range(NCORES))]

    # collectives need whole internal DRAM tensors -- DRAM *pool tiles* are
    # offset views into an arena and break collectives on HW (sim passes).
    cc_in = nc.dram_tensor("cc_in", [NCORES, C + 8], F32)
    cc_out = nc.dram_tensor("cc_out", [NCORES, C + 8], F32)
    st_in = nc.dram_tensor("st_in", [1, 4], F32)
    st_out = nc.dram_tensor("st_out", [NCORES, 4], F32)
    bar_in = nc.dram_tensor("bar_in", [1, 1], F32)
    bar_out = nc.dram_tensor("bar_out", [NCORES, 1], F32)

    with tile.TileContext(nc) as tc:
        with (
            tc.tile_pool(name="const", bufs=1) as cpool,
            tc.tile_pool(name="sb", bufs=1) as sb,
            tc.tile_pool(name="ps", bufs=1, space="PSUM") as ps,
            tc.tile_pool(name="pst", bufs=2, space="PSUM") as pst,
        ):
            # ---- constants (inline Const tensors; avoids gpsimd
            #      standard-library ops that would fight the sparse_gather
            #      library reload for the single Q7 library slot) ---------
            ident_c = nc.inline_tensor(np.eye(P, dtype=np.float32), name="ident_c")
            ident = cpool.tile([P, P], F32, tag="ident")
            nc.sync.dma_start(ident[:], ident_c[:])
            iota_c = nc.inline_tensor(
                np.tile(np.arange(R, dtype=np.float32), (P, 1)), name="iota_c")
            iotaf = cpool.tile([P, R], F32, tag="iotaf")
            nc.sync.dma_start(iotaf[:], iota_c[:])

            # ---- skew-absorbing barrier ------------------------------
            # the 8 NEFF executions start with ~20us of cross-core launch
            # skew; a nearly dependency-free collective issued first absorbs
            # it concurrently with phase 1 so the real AllReduce doesn't wait.
            barz = sb.tile([1, 1], F32, tag="barz")
            nc.vector.memset(barz[:], 0.0)
            nc.sync.dma_start(bar_in[:], barz[:])
            bar_cc = nc.gpsimd.collective_compute(
                "AllGather", AL.bypass, replica_groups=rg,
                ins=[bar_in[:]], outs=[bar_out[:]],
            )

            # ---- big weight loads ------------------------------------
            # background bulk goes on the ACT HWDGE ring (qActDynamicHW)
            # so the critical-path DMAs on the SP ring aren't FIFO-stuck
            # behind 7.5MB of weight streaming.
            wtall = sb.tile([P, CH, HSH], F32, tag="wtall")
            nc.scalar.dma_start(wtall[:], whT_d[:].rearrange("(h p) j -> p h j", p=P))
            wa = sb.tile([P, (R + 1) * CH, C], F32, tag="wa")
            nc.scalar.dma_start(wa[:], waug_d[:].rearrange("(k p) d -> p k d", p=P))
            bh2 = sb.tile([P, Q], F32, tag="bh2")
            nc.scalar.dma_start(bh2[:], bh_d[:])
            mg2 = sb.tile([P, Q], F32, tag="mg2")
            nc.scalar.dma_start(mg2[:], mg_d[:])
            ms2 = sb.tile([P, Q], F32, tag="ms2")
            nc.scalar.dma_start(ms2[:], ms_d[:])
            mgneg2 = sb.tile([P, Q], F32, tag="mgneg2")
            nc.scalar.dma_start(mgneg2[:], mgneg_d[:])
            msneg2 = sb.tile([P, Q], F32, tag="msneg2")
            nc.scalar.dma_start(msneg2[:], msneg_d[:])
            bconv_t = sb.tile([1, C], F32, tag="bconv_t")
            nc.scalar.dma_start(bconv_t[:], bconv_d[:])

            # ---- phase 1: edge scan ----------------------------------
            src_i = sb.tile([P, FREE], I32, tag="src_i")
            nc.sync.dma_start(src_i[:], src_d[:])
            tgt_i = sb.tile([P, FREE], I32, tag="tgt_i")
            nc.sync.dma_start(tgt_i[:], tgt_d[:])
            typ_i = sb.tile([P, FREE], I32, tag="typ_i")
            nc.sync.dma_start(typ_i[:], typ_d[:])

            srcf = sb.tile([P, FREE], F32, tag="srcf")
            nc.vector.tensor_copy(srcf[:], src_i[:])
            tgtf = sb.tile([P, FREE], F32, tag="tgtf")
            nc.vector.tensor_copy(tgtf[:], tgt_i[:])
            typf = sb.tile([P, FREE], F32, tag="typf")
            nc.vector.tensor_copy(typf[:], typ_i[:])

            mask = sb.tile([P, FREE], F32, tag="mask")
            nc.vector.tensor_scalar(
                out=mask[:], in0=tgtf[:], scalar1=0.0, scalar2=None,
                op0=AL.is_equal,
            )
            # enc = (src*8 + type + 1) * mask - 1  -> combined id if the edge
            # targets node 0, else -1.  Exact in f32 (max 400001 < 2^24).
            enc = sb.tile([P, FREE], F32, tag="enc")
            nc.vector.scalar_tensor_tensor(
                out=enc[:], in0=srcf[:], scalar=8.0, in1=typf[:],
                op0=AL.mult, op1=AL.add,
            )
            # t = src*8+typ; matched -> t+1 (>0 even for src=typ=0), else -1:
            # (t+2)*mask - 1
            nc.vector.tensor_scalar_add(enc[:], enc[:], 2.0)
            nc.vector.tensor_tensor(out=enc[:], in0=enc[:], in1=mask[:],
                                    op=AL.mult)
            nc.vector.tensor_scalar_add(enc[:], enc[:], -1.0)

            # per-partition top-8 (native DVE InstMax; the Q7 sparse_gather
            # library op crashes this runtime).  Matched encodes are > 0 and
            # rare (~2 per core over 1024 slots); -1 fills the rest.
            topv = sb.tile([P, 8], F32, tag="topv")
            nc.vector.max(topv[:], enc[:])
            topk = topv[:, 0:KB]     # keep the KB largest (descending order)

            # decode: valid flag, relation, source row
            valid = sb.tile([P, KB], F32, tag="valid")
            nc.vector.tensor_scalar(
                out=valid[:], in0=topk, scalar1=0.0, scalar2=None,
                op0=AL.is_gt,
            )
            dec = sb.tile([P, KB], F32, tag="dec")
            nc.vector.tensor_scalar(
                out=dec[:], in0=topk, scalar1=1.0, scalar2=-1.0,
                op0=AL.max, op1=AL.add,
            )
            # dec = src*8 + typ (exact in f32).  DVE has no mod/floor, so
            # split via int cast with a +-8 correction that is exact under
            # any cast rounding mode.
            u = sb.tile([P, KB], F32, tag="u")
            nc.vector.tensor_scalar_mul(u[:], dec[:], 0.125)
            srcA = sb.tile([P, KB], I32, tag="srcA")
            nc.vector.tensor_copy(srcA[:], u[:])
            srcAf = sb.tile([P, KB], F32, tag="srcAf")
            nc.vector.tensor_copy(srcAf[:], srcA[:])
            err = sb.tile([P, KB], F32, tag="err")
            nc.vector.scalar_tensor_tensor(
                out=err[:], in0=srcAf[:], scalar=-8.0, in1=dec[:],
                op0=AL.mult, op1=AL.add,
            )  # err = dec - 8*srcA  in {typ-8, typ, typ+8}
            adj = sb.tile([P, KB], F32, tag="adj")
            nc.vector.tensor_scalar(
                out=adj[:], in0=err[:], scalar1=0.0, scalar2=8.0,
                op0=AL.is_lt, op1=AL.mult,
            )  # 8 where err<0
            typ8 = sb.tile([P, KB], F32, tag="typ8")
            nc.vector.tensor_tensor(out=typ8[:], in0=err[:], in1=adj[:],
                                    op=AL.add)
            nc.vector.tensor_scalar(
                out=adj[:], in0=typ8[:], scalar1=8.0, scalar2=8.0,
                op0=AL.is_ge, op1=AL.mult,
            )  # 8 where typ8>=8
            nc.vector.tensor_tensor(out=typ8[:], in0=typ8[:], in1=adj[:],
                                    op=AL.subtract)
            src8 = sb.tile([P, KB], F32, tag="src8")
            nc.vector.tensor_tensor(out=src8[:], in0=dec[:], in1=typ8[:],
                                    op=AL.subtract)
            nc.vector.tensor_scalar_mul(src8[:], src8[:], 0.125)
            idx = sb.tile([P, KB], I32, tag="idx")
            idx_cast = nc.vector.tensor_copy(idx[:], src8[:])

            # gather matched x rows (row 0 for empty slots; weight 0).
            # an extra column of ones makes one matmul yield counts too.
            psum_sum = ps.tile([R, C + 1], F32, tag="acc")
            for g in range(KB):
                xg = sb.tile([P, C + 1], F32, tag=f"xg{g}")
                gi = nc.gpsimd.indirect_dma_start(
                    out=xg[:, 0:C],
                    out_offset=None,
                    in_=x_d[:],
                    in_offset=bass.IndirectOffsetOnAxis(ap=idx[:, g:g + 1], axis=0),
                )
                # Tile does not track the offset-AP dependency of indirect
                # DMAs; without this edge the gather races the idx cast.
                tile.add_dep_helper(gi.ins, idx_cast.ins, True,
                                    "gather waits for idx cast")
                nc.vector.memset(xg[:, C:C + 1], 1.0)
                sg = sb.tile([P, R], F32, tag=f"sg{g}")
                nc.vector.tensor_tensor(
                    out=sg[:],
                    in0=typ8[:, g:g + 1].to_broadcast([P, R]),
                    in1=iotaf[:],
                    op=AL.is_equal,
                )
                nc.vector.tensor_tensor(
                    out=sg[:],
                    in0=valid[:, g:g + 1].to_broadcast([P, R]),
                    in1=sg[:],
                    op=AL.mult,
                )
                nc.tensor.matmul(
                    out=psum_sum[:], lhsT=sg[:], rhs=xg[:],
                    start=(g == 0), stop=(g == KB - 1),
                )

            pack = sb.tile([NCORES, C + 8], F32, tag="pack")
            nc.vector.memset(pack[:], 0.0)
            nc.vector.tensor_copy(pack[0:R, 0:C + 1], psum_sum[:])

            # ---- AllReduce of per-relation sums + counts -------------
            nc.sync.dma_start(cc_in[:], pack[:])
            red_cc = nc.gpsimd.collective_compute(
                "AllReduce", AL.add, replica_groups=rg,
                ins=[cc_in[:]], outs=[cc_out[:]],
            )
            tile.add_dep_helper(red_cc.ins, bar_cc.ins, False,
                                "barrier first on the CC queue")
            red = sb.tile([NCORES, C + 8], F32, tag="red")
            nc.sync.dma_start(red[:], cc_out[:])

            # ---- mean + RGCN transform for node 0 --------------------
            cnt_c = sb.tile([R, 1], F32, tag="cnt_c")
            nc.vector.tensor_scalar_max(cnt_c[:], red[0:R, C:C + 1], 1.0)
            rcp = sb.tile([R, 1], F32, tag="rcp")
            nc.vector.reciprocal(rcp[:], cnt_c[:])

            mean_aug = sb.tile([R + 1, C], F32, tag="mean_aug")
            nc.vector.tensor_scalar(
                out=mean_aug[0:R, :], in0=red[0:R, 0:C],
                scalar1=rcp[:], scalar2=None, op0=AL.mult,
            )
            nc.sync.dma_start(mean_aug[R:R + 1, :], x_d[0:1, :])

            # transpose -> [128, 6] x 2 chunks (contraction onto partitions)
            mts = []
            for h in range(CH):
                tp = pst.tile([P, R + 1], F32, tag="tr")
                nc.tensor.transpose(
                    out=tp[:],
                    in_=mean_aug[:, h * P:(h + 1) * P],
                    identity=ident[0:R + 1, 0:R + 1],
                )
                mt = sb.tile([P, R + 1], F32, tag=f"mt{h}")
                nc.vector.tensor_copy(mt[:], tp[:])
                mts.append(mt)

            # h0 = relu(sum_r mean_r @ W_rel[r] + x0 @ W_root + b_conv)
            h0ps = ps.tile([1, C], F32, tag="acc")
            nk = (R + 1) * CH
            for k in range(nk):
                nc.tensor.matmul(
                    out=h0ps[:],
                    lhsT=mts[k % CH][:, k // CH:k // CH + 1],
                    rhs=wa[:, k, :],
                    start=(k == 0), stop=(k == nk - 1),
                )
            h0 = sb.tile([1, C], F32, tag="h0")
            nc.vector.tensor_tensor(
                out=h0[:], in0=h0ps[:], in1=bconv_t[:], op=AL.add
            )
            nc.vector.tensor_scalar_max(h0[:], h0[:], 0.0)

            h0ts = []
            for h in range(CH):
                tp2 = pst.tile([P, 1], F32, tag="tr")
                nc.tensor.transpose(
                    out=tp2[:],
                    in_=h0[0:1, h * P:(h + 1) * P],
                    identity=ident[0:1, 0:1],
                )
                h0t = sb.tile([P, 1], F32, tag=f"h0t{h}")
                nc.vector.tensor_copy(h0t[:], tp2[:])
                h0ts.append(h0t)

            # ---- heads -----------------------------------------------
            # h0T chunks stay stationary on the PE (2 weight loads total);
            # wtall streams through as 512-wide moving blocks.  Each [1,512]
            # psum block jb is repacked into the [128, Q] 2D layout at
            # logits2[p, 4*jb + t] = logit of head row 512*jb + 4*p + t.
            logits2 = sb.tile([P, Q], F32, tag="logits2")
            for jb in range(JB):
                lgj = pst.tile([1, 512], F32, tag="lgj")
                for h in range(CH):
                    nc.tensor.matmul(
                        out=lgj[:],
                        lhsT=h0ts[h][:],
                        rhs=wtall[:, h, jb * 512:(jb + 1) * 512],
                        start=(h == 0), stop=(h == CH - 1),
                    )
                lgs = sb.tile([1, 512], F32, tag=f"lgs{jb % 4}")
                nc.vector.tensor_copy(lgs[:], lgj[:])
                nc.gpsimd.dma_start(
                    logits2[:, 4 * jb:4 * jb + 4], lgs[:]
                )
            nc.vector.tensor_tensor(
                out=logits2[:], in0=logits2[:], in1=bh2[:], op=AL.add
            )

            # ---- per-core log-softmax partials (flash style) ---------
            stats = sb.tile([1, 4], F32, tag="stats")
            for hi, (m2, mn2) in enumerate(((mg2, mgneg2), (ms2, msneg2))):
                masked = sb.tile([P, Q], F32, tag=f"masked{hi}")
                nc.vector.tensor_tensor(
                    out=masked[:], in0=logits2[:], in1=m2[:], op=AL.mult
                )
                nc.vector.tensor_tensor(
                    out=masked[:], in0=masked[:], in1=mn2[:], op=AL.add
                )
                pmax = sb.tile([P, 1], F32, tag=f"pmax{hi}")
                nc.vector.reduce_max(pmax[:], masked[:], axis=AX.X)
                npmax = sb.tile([P, 1], F32, tag=f"npmax{hi}")
                nc.vector.tensor_scalar_mul(npmax[:], pmax[:], -1.0)
                exps = sb.tile([P, Q], F32, tag=f"exps{hi}")
                esum = sb.tile([P, 1], F32, tag=f"esum{hi}")
                nc.scalar.activation(
                    out=exps[:], in_=masked[:], func=ACT.Exp,
                    bias=npmax[:], scale=1.0, accum_out=esum[:],
                )
                # fold 128 per-partition (m, s) pairs down to one (M, S)
                tpm = pst.tile([1, P], F32, tag="trv")
                nc.tensor.transpose(out=tpm[:], in_=pmax[:], identity=ident[:])
                mrow = sb.tile([1, P], F32, tag=f"mrow{hi}")
                nc.vector.tensor_copy(mrow[:], tpm[:])
                tps = pst.tile([1, P], F32, tag="trv")
                nc.tensor.transpose(out=tps[:], in_=esum[:], identity=ident[:])
                srow = sb.tile([1, P], F32, tag=f"srow{hi}")
                nc.vector.tensor_copy(srow[:], tps[:])

                Mh = sb.tile([1, 1], F32, tag=f"Mh{hi}")
                nc.vector.reduce_max(Mh[:], mrow[:], axis=AX.X)
                nMh = sb.tile([1, 1], F32, tag=f"nMh{hi}")
                nc.vector.tensor_scalar_mul(nMh[:], Mh[:], -1.0)
                wex = sb.tile([1, P], F32, tag=f"wex{hi}")
                nc.scalar.activation(
                    out=wex[:], in_=mrow[:], func=ACT.Exp, bias=nMh[:], scale=1.0
                )
                sw = sb.tile([1, P], F32, tag=f"sw{hi}")
                nc.vector.tensor_tensor(out=sw[:], in0=wex[:], in1=srow[:], op=AL.mult)
                Sh = sb.tile([1, 1], F32, tag=f"Sh{hi}")
                nc.vector.reduce_sum(Sh[:], sw[:], axis=AX.X)

                nc.vector.tensor_copy(stats[0:1, 2 * hi:2 * hi + 1], Mh[:])
                nc.vector.tensor_copy(stats[0:1, 2 * hi + 1:2 * hi + 2], Sh[:])

            # ---- AllGather stats, final LSE per head -----------------
            nc.sync.dma_start(st_in[:], stats[:])
            nc.gpsimd.collective_compute(
                "AllGather", AL.bypass, replica_groups=rg,
                ins=[st_in[:]], outs=[st_out[:]],
            )
            stb = sb.tile([NCORES, 4], F32, tag="stb")
            nc.sync.dma_start(stb[:], st_out[:])
            statsf = sb.tile([1, 4 * NCORES], F32, tag="statsf")
            nc.gpsimd.dma_start(statsf[:], stb[:])
            # statsf[0, 4i + c]: core i, c in (Mg, Sg, Ms, Ss)

            lse_pair = sb.tile([1, 2], F32, tag="lse_pair")
            dbg_t = sb.tile([1, 8], F32, tag="dbg_t")
            nc.vector.memset(dbg_t[:], 0.0)
            for hi in range(2):
                mvals = statsf[0:1, 2 * hi::4]       # [1, 8]
                svals = statsf[0:1, 2 * hi + 1::4]   # [1, 8]
                MG = sb.tile([1, 1], F32, tag=f"MG{hi}")
                nc.vector.reduce_max(MG[:], mvals, axis=AX.X)
                nMG = sb.tile([1, 1], F32, tag=f"nMG{hi}")
                nc.vector.tensor_scalar_mul(nMG[:], MG[:], -1.0)
                wex2 = sb.tile([1, NCORES], F32, tag=f"wex2{hi}")
                nc.scalar.activation(
                    out=wex2[:], in_=mvals, func=ACT.Exp, bias=nMG[:], scale=1.0
                )
                sw2 = sb.tile([1, NCORES], F32, tag=f"sw2{hi}")
                nc.vector.tensor_tensor(out=sw2[:], in0=wex2[:], in1=svals, op=AL.mult)
                SG = sb.tile([1, 1], F32, tag=f"SG{hi}")
                nc.vector.reduce_sum(SG[:], sw2[:], axis=AX.X)
                lnS = sb.tile([1, 1], F32, tag=f"lnS{hi}")
                nc.scalar.activation(
                    out=lnS[:], in_=SG[:], func=ACT.Ln, bias=0.0, scale=1.0
                )
                nc.vector.tensor_tensor(
                    out=lse_pair[0:1, hi:hi + 1], in0=MG[:], in1=lnS[:], op=AL.add
                )
                nc.vector.tensor_copy(dbg_t[0:1, 2 + hi:3 + hi], MG[:])

            # broadcast the two LSE scalars to all 128 partitions via PE
            ones_row = cpool.tile([1, P], F32, tag="ones_row")
            nc.vector.memset(ones_row[:], 1.0)
            bc_ps = pst.tile([P, 2], F32, tag="tr")
            nc.tensor.matmul(
                out=bc_ps[:], lhsT=ones_row[:], rhs=lse_pair[:],
                start=True, stop=True,
            )
            lseb = sb.tile([P, 2], F32, tag="lseb")
            nc.vector.tensor_copy(lseb[:], bc_ps[:])

            # out = logits - mg*LSE_g - ms*LSE_s
            t1 = sb.tile([P, Q], F32, tag="t1")
            nc.vector.tensor_scalar(
                out=t1[:], in0=mg2[:], scalar1=lseb[:, 0:1], scalar2=None,
                op0=AL.mult,
            )
            t2 = sb.tile([P, Q], F32, tag="t2")
            nc.vector.tensor_scalar(
                out=t2[:], in0=ms2[:], scalar1=lseb[:, 1:2], scalar2=None,
                op0=AL.mult,
            )
            nc.vector.tensor_tensor(out=t1[:], in0=t1[:], in1=t2[:], op=AL.add)
            out2 = sb.tile([P, Q], F32, tag="out2")
            nc.vector.tensor_tensor(
                out=out2[:], in0=logits2[:], in1=t1[:], op=AL.subtract
            )
            nc.sync.dma_start(out_d[:], out2[:])

            # debug output: total matched-edge count for this core; also
            # consume bar_out so the barrier collective isn't dead-code.
            nvalid = sb.tile([P, 1], F32, tag="nvalid")
            nc.vector.reduce_sum(nvalid[:], valid[:], axis=AX.X)
            tnv = pst.tile([1, P], F32, tag="trv")
            nc.tensor.transpose(out=tnv[:], in_=nvalid[:], identity=ident[:])
            nvrow = sb.tile([1, P], F32, tag="nvrow")
            nc.vector.tensor_copy(nvrow[:], tnv[:])
            nc.vector.reduce_sum(dbg_t[0:1, 0:1], nvrow[:], axis=AX.X)
            bar_sb = sb.tile([1, 1], F32, tag="bar_sb")
            nc.sync.dma_start(bar_sb[:], bar_out[0:1, 0:1])
            nc.vector.tensor_copy(dbg_t[0:1, 7:8], bar_sb[:])
            nc.sync.dma_start(dbg_d[:], dbg_t[:])

    nc.compile()
    return nc


def _shuffle_head_vec(vec: np.ndarray) -> np.ndarray:
    """[HSH] -> [P, Q] with head row 512*jb + 4*p + t at [p, 4*jb + t]."""
    return np.ascontiguousarray(
        vec.reshape(JB, P, 4).transpose(1, 0, 2).reshape(P, Q))


def make_in_maps(x, edge_index, edge_type, W_rel, W_root, b_conv,
                 W_global, b_global, W_sense, b_sense):
    x = np.ascontiguousarray(np.asarray(x, dtype=np.float32))
    src_all = np.asarray(edge_index[0], dtype=np.int32)
    tgt_all = np.asarray(edge_index[1], dtype=np.int32)
    typ_all = np.asarray(edge_type, dtype=np.int32)

    waug = np.ascontiguousarray(np.concatenate(
        [np.asarray(W_rel, dtype=np.float32).reshape(R * C, C),
         np.asarray(W_root, dtype=np.float32)], axis=0))
    bconv = np.ascontiguousarray(
        np.asarray(b_conv, dtype=np.float32).reshape(1, C))

    w_head = np.concatenate(
        [np.asarray(W_global, dtype=np.float32),
         np.asarray(W_sense, dtype=np.float32),
         np.zeros((HEAD_PAD - HEAD, C), dtype=np.float32)], axis=0)
    w_head_T = np.ascontiguousarray(w_head.T)          # [C, HEAD_PAD]
    bh_vec = np.concatenate(
        [np.asarray(b_global, dtype=np.float32),
         np.asarray(b_sense, dtype=np.float32),
         np.zeros(HEAD_PAD - HEAD, dtype=np.float32)])
    jab = np.arange(HEAD_PAD)
    mg_vec = (jab < GLOBALS).astype(np.float32)
    ms_vec = ((jab >= GLOBALS) & (jab < HEAD)).astype(np.float32)
    mgneg_vec = (1.0 - mg_vec) * NEG
    msneg_vec = (1.0 - ms_vec) * NEG

    in_maps = []
    for i in range(NCORES):
        sl = slice(i * ESH, (i + 1) * ESH)
        pad = SH_PAD - ESH
        src_i = np.concatenate([src_all[sl], np.zeros(pad, np.int32)])
        tgt_i = np.concatenate([tgt_all[sl], np.ones(pad, np.int32)])
        typ_i = np.concatenate([typ_all[sl], np.zeros(pad, np.int32)])
        hs = slice(i * HSH, (i + 1) * HSH)
        in_maps.append({
            "x": x,
            "src": np.ascontiguousarray(src_i.reshape(P, FREE)),
            "tgt": np.ascontiguousarray(tgt_i.reshape(P, FREE)),
            "typ": np.ascontiguousarray(typ_i.reshape(P, FREE)),
            "waug": waug,
            "bconv": bconv,
            "whT": np.ascontiguousarray(w_head_T[:, hs]),
            "bh": _shuffle_head_vec(bh_vec[hs]),
            "mg": _shuffle_head_vec(mg_vec[hs]),
            "ms": _shuffle_head_vec(ms_vec[hs]),
            "mgneg": _shuffle_head_vec(mgneg_vec[hs]),
            "msneg": _shuffle_head_vec(msneg_vec[hs]),
        })
    return in_maps


def postprocess(outs: list) -> tuple:
    flat = np.concatenate([
        np.asarray(o).reshape(P, JB, 4).transpose(1, 0, 2).reshape(-1)
        for o in outs])
    return (np.ascontiguousarray(flat[:GLOBALS], dtype=np.float32),
            np.ascontiguousarray(flat[GLOBALS:HEAD], dtype=np.float32))


_NC_CACHE = None


def kernel(**inputs) -> tuple:
    global _NC_CACHE
    if _NC_CACHE is None:
        _NC_CACHE = build_program()
    nc = _NC_CACHE
    in_maps = make_in_maps(**inputs)
    res = run_bass_kernel_spmd(nc, in_maps, list(range(NCORES)))
    return postprocess([r["out"] for r in res.results])


# revision 49
# speedup vs baseline: 1.1915x; 1.0134x over previous
"""Trainium2 Bass kernel for nn_NetRGCN (RGCN conv -> two linear heads on node 0).

Key structure: the module's output is
    (log_softmax(W_global @ h0 + b_global), log_softmax(W_sense @ h0 + b_sense))
with h0 = relu(rgcn_conv(x, ...)[0]).  Under per-(target-node, relation) mean
aggregation, h0 depends only on edges whose TARGET is node 0 (~E/N of the 800k
edges) plus x[0] @ W_root.  The kernel:

  phase 1 (edges sharded 8 ways): each core scans its 100k-edge shard
    on-device, compacts the (src, type) pairs of edges targeting node 0 with
    gpsimd sparse_gather, indirect-DMA-gathers just those x rows, and reduces
    them into per-relation sums + counts with relation-onehot PE matmuls.
  AllReduce: [5, 257] per-relation sums+counts across the 8 cores.
  phase 2: every core computes mean -> per-relation transform + root term ->
    h0 (tiny, replicated), then its 5632-row slice of the stacked padded
    [45056, 256] head matrix as PE matvecs plus log-softmax partial stats.
  AllGather: 4 stats floats per core; each core normalizes its slice of the
    final log-probs on device.

Host work is limited to slicing/padding/transposing inputs and concatenating
the 8 output slices.
"""

import numpy as np

import concourse.bass as bass
import concourse.bacc as bacc
import concourse.mybir as mybir
import concourse.tile as tile
from concourse.bass_utils import run_bass_kernel_spmd

F32 = mybir.dt.float32
I32 = mybir.dt.int32
U32 = mybir.dt.uint32
AL = mybir.AluOpType
ACT = mybir.ActivationFunctionType
AX = mybir.AxisListType

# problem sizes (hardcoded per contract)
N, C, R = 50000, 256, 5
E = 800000
GLOBALS, SENSES = 25000, 20000
HEAD = GLOBALS + SENSES          # 45000
NCORES = 8
P = 128

ESH = E // NCORES                # 100000 edges per core
FREE = 784                       # 128*784 = 100352 padded shard
SH_PAD = P * FREE
KB = 4                           # matched edges kept per partition (of the
                                 # DVE top-8) -> 4 gather batches of 128

HEAD_PAD = 45056                 # 8 * 5632
HSH = HEAD_PAD // NCORES         # 5632 head rows per core
Q = HSH // P                     # 44 columns in the [128, 44] 2D layout
JB = HSH // 512                  # 11 psum blocks of 512 head rows
CH = C // P                      # 2 contraction chunks of 128

NEG = -1.0e30


def build_program() -> bass.Bass:
    # Bacc (not raw Bass): its compile() pass legalizes multi-wait sync via
    # event semaphores, auto-inserts gpsimd library loads for sparse_gather,
    # and encodes extended-ISA instruction bytes -- all required by walrus.
    nc = bacc.Bacc("TRN2", target_bir_lowering=False, debug=False,
                   num_devices=NCORES)

    # ---- per-core I/O -------------------------------------------------
    x_d = nc.declare_dram_parameter("x", [N, C], F32, isOutput=False)
    src_d = nc.declare_dram_parameter("src", [P, FREE], I32, isOutput=False)
    tgt_d = nc.declare_dram_parameter("tgt", [P, FREE], I32, isOutput=False)
    typ_d = nc.declare_dram_parameter("typ", [P, FREE], I32, isOutput=False)
    waug_d = nc.declare_dram_parameter("waug", [(R + 1) * C, C], F32, isOutput=False)
    bconv_d = nc.declare_dram_parameter("bconv", [P, CH], F32, isOutput=False)
    whT_d = nc.declare_dram_parameter("whT", [C, HSH], F32, isOutput=False)
    bh_d = nc.declare_dram_parameter("bh", [P, Q], F32, isOutput=False)
    mg_d = nc.declare_dram_parameter("mg", [P, Q], F32, isOutput=False)
    ms_d = nc.declare_dram_parameter("ms", [P, Q], F32, isOutput=False)
    mgneg_d = nc.declare_dram_parameter("mgneg", [P, Q], F32, isOutput=False)
    msneg_d = nc.declare_dram_parameter("msneg", [P, Q], F32, isOutput=False)

    out_d = nc.declare_dram_parameter("out", [P, Q], F32, isOutput=True)
    dbg_d = nc.declare_dram_parameter("dbg", [1, 8], F32, isOutput=True)

    rg = [list(range(NCORES))]

    # collectives need whole internal DRAM tensors -- DRAM *pool tiles* are
    # offset views into an arena and break collectives on HW (sim passes).
    cc_in = nc.dram_tensor("cc_in", [P, 18], F32)
    cc_out = nc.dram_tensor("cc_out", [P, 18], F32)
    st_in = nc.dram_tensor("st_in", [1, 4], F32)
    st_out = nc.dram_tensor("st_out", [NCORES, 4], F32)

    with tile.TileContext(nc) as tc:
        with (
            tc.tile_pool(name="const", bufs=1) as cpool,
            tc.tile_pool(name="sb", bufs=1) as sb,
            tc.tile_pool(name="ps", bufs=1, space="PSUM") as ps,
            tc.tile_pool(name="pst", bufs=2, space="PSUM") as pst,
            tc.tile_pool(name="psq", bufs=3, space="PSUM") as psq,
        ):
            # ---- constants (inline Const tensors; avoids gpsimd
            #      standard-library ops that would fight the sparse_gather
            #      library reload for the single Q7 library slot) ---------
            ident_c = nc.inline_tensor(np.eye(P, dtype=np.float32), name="ident_c")
            ident = cpool.tile([P, P], F32, tag="ident")
            nc.sync.dma_start(ident[:], ident_c[:])
            iota_c = nc.inline_tensor(
                np.tile(np.arange(R, dtype=np.float32), (P, 1)), name="iota_c")
            iotaf = cpool.tile([P, R], F32, tag="iotaf")
            nc.sync.dma_start(iotaf[:], iota_c[:])
            ones_row = cpool.tile([1, P], F32, tag="ones_row")
            nc.vector.memset(ones_row[:], 1.0)

            # ---- big weight loads ------------------------------------
            # background bulk goes on the ACT HWDGE ring (qActDynamicHW)
            # so the critical-path DMAs on the SP ring aren't FIFO-stuck
            # behind 7.5MB of weight streaming.
            wtall = sb.tile([P, CH, HSH], F32, tag="wtall")
            nc.scalar.dma_start(wtall[:], whT_d[:].rearrange("(h p) j -> p h j", p=P))
            wa = sb.tile([P, (R + 1) * CH, C], F32, tag="wa")
            nc.scalar.dma_start(wa[:], waug_d[:].rearrange("(k p) d -> p k d", p=P))
            bh2 = sb.tile([P, Q], F32, tag="bh2")
            nc.scalar.dma_start(bh2[:], bh_d[:])
            mg2 = sb.tile([P, Q], F32, tag="mg2")
            nc.scalar.dma_start(mg2[:], mg_d[:])
            ms2 = sb.tile([P, Q], F32, tag="ms2")
            nc.scalar.dma_start(ms2[:], ms_d[:])
            mgneg2 = sb.tile([P, Q], F32, tag="mgneg2")
            nc.scalar.dma_start(mgneg2[:], mgneg_d[:])
            msneg2 = sb.tile([P, Q], F32, tag="msneg2")
            nc.scalar.dma_start(msneg2[:], msneg_d[:])
            bconv2_t = sb.tile([P, CH], F32, tag="bconv2_t")
            nc.scalar.dma_start(bconv2_t[:], bconv_d[:])

            # ---- phase 1: edge scan ----------------------------------
            src_i = sb.tile([P, FREE], I32, tag="src_i")
            nc.sync.dma_start(src_i[:], src_d[:])
            tgt_i = sb.tile([P, FREE], I32, tag="tgt_i")
            nc.sync.dma_start(tgt_i[:], tgt_d[:])
            typ_i = sb.tile([P, FREE], I32, tag="typ_i")
            nc.sync.dma_start(typ_i[:], typ_d[:])

            srcf = sb.tile([P, FREE], F32, tag="srcf")
            nc.vector.tensor_copy(srcf[:], src_i[:])
            tgtf = sb.tile([P, FREE], F32, tag="tgtf")
            nc.vector.tensor_copy(tgtf[:], tgt_i[:])
            typf = sb.tile([P, FREE], F32, tag="typf")
            nc.vector.tensor_copy(typf[:], typ_i[:])

            mask = sb.tile([P, FREE], F32, tag="mask")
            nc.vector.tensor_scalar(
                out=mask[:], in0=tgtf[:], scalar1=0.0, scalar2=None,
                op0=AL.is_equal,
            )
            # enc = (src*8 + type + 1) * mask - 1  -> combined id if the edge
            # targets node 0, else -1.  Exact in f32 (max 400001 < 2^24).
            enc = sb.tile([P, FREE], F32, tag="enc")
            nc.vector.scalar_tensor_tensor(
                out=enc[:], in0=srcf[:], scalar=8.0, in1=typf[:],
                op0=AL.mult, op1=AL.add,
            )
            # t = src*8+typ; matched -> t+1 (>0 even for src=typ=0), else -1:
            # (t+2)*mask - 1
            nc.vector.tensor_scalar_add(enc[:], enc[:], 2.0)
            nc.vector.tensor_tensor(out=enc[:], in0=enc[:], in1=mask[:],
                                    op=AL.mult)
            nc.vector.tensor_scalar_add(enc[:], enc[:], -1.0)

            # per-partition top-8 (native DVE InstMax; the Q7 sparse_gather
            # library op crashes this runtime).  Matched encodes are > 0 and
            # rare (~2 per core over 1024 slots); -1 fills the rest.
            topv = sb.tile([P, 8], F32, tag="topv")
            nc.vector.max(topv[:], enc[:])
            topk = topv[:, 0:KB]     # keep the KB largest (descending order)

            # decode: valid flag, relation, source row
            valid = sb.tile([P, KB], F32, tag="valid")
            nc.vector.tensor_scalar(
                out=valid[:], in0=topk, scalar1=0.0, scalar2=None,
                op0=AL.is_gt,
            )
            dec = sb.tile([P, KB], F32, tag="dec")
            nc.vector.tensor_scalar(
                out=dec[:], in0=topk, scalar1=1.0, scalar2=-1.0,
                op0=AL.max, op1=AL.add,
            )
            # dec = src*8 + typ (exact in f32).  DVE has no mod/floor, so
            # split via int cast with a +-8 correction that is exact under
            # any cast rounding mode.
            u = sb.tile([P, KB], F32, tag="u")
            nc.vector.tensor_scalar_mul(u[:], dec[:], 0.125)
            srcA = sb.tile([P, KB], I32, tag="srcA")
            nc.vector.tensor_copy(srcA[:], u[:])
            srcAf = sb.tile([P, KB], F32, tag="srcAf")
            nc.vector.tensor_copy(srcAf[:], srcA[:])
            err = sb.tile([P, KB], F32, tag="err")
            nc.vector.scalar_tensor_tensor(
                out=err[:], in0=srcAf[:], scalar=-8.0, in1=dec[:],
                op0=AL.mult, op1=AL.add,
            )  # err = dec - 8*srcA  in {typ-8, typ, typ+8}
            adj = sb.tile([P, KB], F32, tag="adj")
            nc.vector.tensor_scalar(
                out=adj[:], in0=err[:], scalar1=0.0, scalar2=8.0,
                op0=AL.is_lt, op1=AL.mult,
            )  # 8 where err<0
            typ8 = sb.tile([P, KB], F32, tag="typ8")
            nc.vector.tensor_tensor(out=typ8[:], in0=err[:], in1=adj[:],
                                    op=AL.add)
            nc.vector.tensor_scalar(
                out=adj[:], in0=typ8[:], scalar1=8.0, scalar2=8.0,
                op0=AL.is_ge, op1=AL.mult,
            )  # 8 where typ8>=8
            nc.vector.tensor_tensor(out=typ8[:], in0=typ8[:], in1=adj[:],
                                    op=AL.subtract)
            src8 = sb.tile([P, KB], F32, tag="src8")
            nc.vector.tensor_tensor(out=src8[:], in0=dec[:], in1=typ8[:],
                                    op=AL.subtract)
            nc.vector.tensor_scalar_mul(src8[:], src8[:], 0.125)
            idx = sb.tile([P, KB], I32, tag="idx")
            idx_cast = nc.vector.tensor_copy(idx[:], src8[:])

            # gather matched x rows (row 0 for empty slots; weight 0).
            # an extra column of ones makes one matmul yield counts too.
            psum_sum = ps.tile([R, C + 1], F32, tag="acc")
            for g in range(KB):
                xg = sb.tile([P, C + 1], F32, tag=f"xg{g}")
                gi = nc.gpsimd.indirect_dma_start(
                    out=xg[:, 0:C],
                    out_offset=None,
                    in_=x_d[:],
                    in_offset=bass.IndirectOffsetOnAxis(ap=idx[:, g:g + 1], axis=0),
                )
                # Tile does not track the offset-AP dependency of indirect
                # DMAs; without this edge the gather races the idx cast.
                tile.add_dep_helper(gi.ins, idx_cast.ins, True,
                                    "gather waits for idx cast")
                nc.vector.memset(xg[:, C:C + 1], 1.0)
                sg = sb.tile([P, R], F32, tag=f"sg{g}")
                nc.vector.tensor_tensor(
                    out=sg[:],
                    in0=typ8[:, g:g + 1].to_broadcast([P, R]),
                    in1=iotaf[:],
                    op=AL.is_equal,
                )
                nc.vector.tensor_tensor(
                    out=sg[:],
                    in0=valid[:, g:g + 1].to_broadcast([P, R]),
                    in1=sg[:],
                    op=AL.mult,
                )
                nc.tensor.matmul(
                    out=psum_sum[:], lhsT=sg[:], rhs=xg[:],
                    start=(g == 0), stop=(g == KB - 1),
                )

            # ---- pre-collective RGCN transform -----------------------
            # The per-relation division by the (global) count commutes with
            # the linear maps, so each core transforms its PARTIAL sums first
            # and the AllReduce carries already-transformed d-space vectors:
            #   z = sum_r (sum_i sum0_i,r @ W_rel[r]) / cnt_r + x0@W_root + b
            # yT layout: [128(d within half dh), 6] with col r<5 = relation r
            # contribution, col 5 = (x0 @ W_root)/8 (every core adds 1/8th).
            s0 = sb.tile([R, C + 1], F32, tag="s0")
            nc.vector.tensor_copy(s0[:], psum_sum[:])
            x0sb = sb.tile([1, C], F32, tag="x0sb")
            nc.sync.dma_start(x0sb[:], x_d[0:1, :])
            s0ts = []
            for h in range(CH):
                tp = pst.tile([P, R], F32, tag="tr")
                nc.tensor.transpose(
                    out=tp[:],
                    in_=s0[0:R, h * P:(h + 1) * P],
                    identity=ident[0:R, 0:R],
                )
                s0t = sb.tile([P, R + 1], F32, tag=f"s0t{h}")
                nc.vector.tensor_copy(s0t[:, 0:R], tp[:])
                tp2 = pst.tile([P, 1], F32, tag="tr")
                nc.tensor.transpose(
                    out=tp2[:],
                    in_=x0sb[0:1, h * P:(h + 1) * P],
                    identity=ident[0:1, 0:1],
                )
                nc.vector.tensor_scalar_mul(s0t[:, R:R + 1], tp2[:], 0.125)
                s0ts.append(s0t)
            # counts -> row vector [1, R]
            cntr_ps = pst.tile([1, R], F32, tag="tr")
            nc.tensor.transpose(
                out=cntr_ps[:], in_=s0[0:R, C:C + 1],
                identity=ident[0:R, 0:R],
            )
            pack = sb.tile([P, 18], F32, tag="pack")
            nc.vector.memset(pack[:], 0.0)
            nc.vector.tensor_copy(pack[0:1, 12:12 + R], cntr_ps[:])

            for dh in range(CH):
                yt = ps.tile([P, R + 1], F32, tag="acc")
                for col in range(R + 1):
                    for h in range(CH):
                        nc.tensor.matmul(
                            out=yt[:, col:col + 1],
                            lhsT=wa[:, 2 * col + h, dh * P:(dh + 1) * P],
                            rhs=s0ts[h][:, col:col + 1],
                            start=(h == 0), stop=(h == CH - 1),
                        )
                nc.vector.tensor_copy(pack[:, 6 * dh:6 * dh + 6], yt[:])

            # ---- AllReduce of transformed vectors + counts -----------
            nc.sync.dma_start(cc_in[:], pack[:])
            nc.gpsimd.collective_compute(
                "AllReduce", AL.add, replica_groups=rg,
                ins=[cc_in[:]], outs=[cc_out[:]],
            )
            red = sb.tile([P, 18], F32, tag="red")
            nc.sync.dma_start(red[:], cc_out[:])

            # ---- combine into h0T halves -----------------------------
            # per-relation reciprocal counts (col 5 -> 1.0 for the root
            # term), broadcast down partitions via a rank-1 matmul.
            cnt6 = sb.tile([1, R + 1], F32, tag="cnt6")
            nc.vector.tensor_scalar_max(cnt6[0:1, 0:R], red[0:1, 12:12 + R], 1.0)
            nc.vector.memset(cnt6[0:1, R:R + 1], 1.0)
            rcp6 = sb.tile([1, R + 1], F32, tag="rcp6")
            nc.vector.reciprocal(rcp6[:], cnt6[:])
            bc_ps6 = pst.tile([P, R + 1], F32, tag="tr")
            nc.tensor.matmul(
                out=bc_ps6[:], lhsT=ones_row[:], rhs=rcp6[:],
                start=True, stop=True,
            )
            rcpb6 = sb.tile([P, R + 1], F32, tag="rcpb6")
            nc.vector.tensor_copy(rcpb6[:], bc_ps6[:])

            h0ts = []
            for dh in range(CH):
                zt = sb.tile([P, R + 1], F32, tag=f"zt{dh}")
                nc.vector.tensor_tensor(
                    out=zt[:], in0=red[:, 6 * dh:6 * dh + 6], in1=rcpb6[:],
                    op=AL.mult,
                )
                h0t = sb.tile([P, 1], F32, tag=f"h0t{dh}")
                nc.vector.reduce_sum(h0t[:], zt[:], axis=AX.X)
                nc.vector.tensor_tensor(
                    out=h0t[:], in0=h0t[:], in1=bconv2_t[:, dh:dh + 1],
                    op=AL.add,
                )
                nc.vector.tensor_scalar_max(h0t[:], h0t[:], 0.0)
                h0ts.append(h0t)

            # ---- heads -----------------------------------------------
            # h0T chunks stay stationary on the PE (2 weight loads total);
            # wtall streams through as 512-wide moving blocks.  Each [1,512]
            # psum block jb is repacked into the [128, Q] 2D layout at
            # logits2[p, 4*jb + t] = logit of head row 512*jb + 4*p + t.
            logits2 = sb.tile([P, Q], F32, tag="logits2")
            for jb in range(JB):
                lgj = psq.tile([1, 512], F32, tag="lgj")
                for h in range(CH):
                    nc.tensor.matmul(
                        out=lgj[:],
                        lhsT=h0ts[h][:],
                        rhs=wtall[:, h, jb * 512:(jb + 1) * 512],
                        start=(h == 0), stop=(h == CH - 1),
                    )
                lgs = sb.tile([1, 512], F32, tag=f"lgs{jb % 4}")
                nc.vector.tensor_copy(lgs[:], lgj[:])
                nc.gpsimd.dma_start(
                    logits2[:, 4 * jb:4 * jb + 4], lgs[:]
                )
            nc.vector.tensor_tensor(
                out=logits2[:], in0=logits2[:], in1=bh2[:], op=AL.add
            )

            # ---- per-core log-softmax partials (flash style) ---------
            stats = sb.tile([1, 4], F32, tag="stats")
            for hi, (m2, mn2) in enumerate(((mg2, mgneg2), (ms2, msneg2))):
                masked = sb.tile([P, Q], F32, tag=f"masked{hi}")
                nc.vector.tensor_tensor(
                    out=masked[:], in0=logits2[:], in1=m2[:], op=AL.mult
                )
                nc.vector.tensor_tensor(
                    out=masked[:], in0=masked[:], in1=mn2[:], op=AL.add
                )
                pmax = sb.tile([P, 1], F32, tag=f"pmax{hi}")
                nc.vector.reduce_max(pmax[:], masked[:], axis=AX.X)
                npmax = sb.tile([P, 1], F32, tag=f"npmax{hi}")
                nc.vector.tensor_scalar_mul(npmax[:], pmax[:], -1.0)
                exps = sb.tile([P, Q], F32, tag=f"exps{hi}")
                esum = sb.tile([P, 1], F32, tag=f"esum{hi}")
                nc.scalar.activation(
                    out=exps[:], in_=masked[:], func=ACT.Exp,
                    bias=npmax[:], scale=1.0, accum_out=esum[:],
                )
                # fold 128 per-partition (m, s) pairs down to one (M, S)
                tpm = pst.tile([1, P], F32, tag="trv")
                nc.tensor.transpose(out=tpm[:], in_=pmax[:], identity=ident[:])
                mrow = sb.tile([1, P], F32, tag=f"mrow{hi}")
                nc.vector.tensor_copy(mrow[:], tpm[:])
                tps = pst.tile([1, P], F32, tag="trv")
                nc.tensor.transpose(out=tps[:], in_=esum[:], identity=ident[:])
                srow = sb.tile([1, P], F32, tag=f"srow{hi}")
                nc.vector.tensor_copy(srow[:], tps[:])

                Mh = sb.tile([1, 1], F32, tag=f"Mh{hi}")
                nc.vector.reduce_max(Mh[:], mrow[:], axis=AX.X)
                nMh = sb.tile([1, 1], F32, tag=f"nMh{hi}")
                nc.vector.tensor_scalar_mul(nMh[:], Mh[:], -1.0)
                wex = sb.tile([1, P], F32, tag=f"wex{hi}")
                nc.scalar.activation(
                    out=wex[:], in_=mrow[:], func=ACT.Exp, bias=nMh[:], scale=1.0
                )
                sw = sb.tile([1, P], F32, tag=f"sw{hi}")
                nc.vector.tensor_tensor(out=sw[:], in0=wex[:], in1=srow[:], op=AL.mult)
                Sh = sb.tile([1, 1], F32, tag=f"Sh{hi}")
                nc.vector.reduce_sum(Sh[:], sw[:], axis=AX.X)

                nc.vector.tensor_copy(stats[0:1, 2 * hi:2 * hi + 1], Mh[:])
                nc.vector.tensor_copy(stats[0:1, 2 * hi + 1:2 * hi + 2], Sh[:])

            # ---- AllGather stats, final LSE per head -----------------
            nc.sync.dma_start(st_in[:], stats[:])
            nc.gpsimd.collective_compute(
                "AllGather", AL.bypass, replica_groups=rg,
                ins=[st_in[:]], outs=[st_out[:]],
            )
            stb = sb.tile([NCORES, 4], F32, tag="stb")
            nc.sync.dma_start(stb[:], st_out[:])
            statsf = sb.tile([1, 4 * NCORES], F32, tag="statsf")
            nc.gpsimd.dma_start(statsf[:], stb[:])
            # statsf[0, 4i + c]: core i, c in (Mg, Sg, Ms, Ss)

            lse_pair = sb.tile([1, 2], F32, tag="lse_pair")
            dbg_t = sb.tile([1, 8], F32, tag="dbg_t")
            nc.vector.memset(dbg_t[:], 0.0)
            for hi in range(2):
                mvals = statsf[0:1, 2 * hi::4]       # [1, 8]
                svals = statsf[0:1, 2 * hi + 1::4]   # [1, 8]
                MG = sb.tile([1, 1], F32, tag=f"MG{hi}")
                nc.vector.reduce_max(MG[:], mvals, axis=AX.X)
                nMG = sb.tile([1, 1], F32, tag=f"nMG{hi}")
                nc.vector.tensor_scalar_mul(nMG[:], MG[:], -1.0)
                wex2 = sb.tile([1, NCORES], F32, tag=f"wex2{hi}")
                nc.scalar.activation(
                    out=wex2[:], in_=mvals, func=ACT.Exp, bias=nMG[:], scale=1.0
                )
                sw2 = sb.tile([1, NCORES], F32, tag=f"sw2{hi}")
                nc.vector.tensor_tensor(out=sw2[:], in0=wex2[:], in1=svals, op=AL.mult)
                SG = sb.tile([1, 1], F32, tag=f"SG{hi}")
                nc.vector.reduce_sum(SG[:], sw2[:], axis=AX.X)
                lnS = sb.tile([1, 1], F32, tag=f"lnS{hi}")
                nc.scalar.activation(
                    out=lnS[:], in_=SG[:], func=ACT.Ln, bias=0.0, scale=1.0
                )
                nc.vector.tensor_tensor(
                    out=lse_pair[0:1, hi:hi + 1], in0=MG[:], in1=lnS[:], op=AL.add
                )
                nc.vector.tensor_copy(dbg_t[0:1, 2 + hi:3 + hi], MG[:])

            # broadcast the two LSE scalars to all 128 partitions via PE
            bc_ps = pst.tile([P, 2], F32, tag="tr")
            nc.tensor.matmul(
                out=bc_ps[:], lhsT=ones_row[:], rhs=lse_pair[:],
                start=True, stop=True,
            )
            lseb = sb.tile([P, 2], F32, tag="lseb")
            nc.vector.tensor_copy(lseb[:], bc_ps[:])

            # out = logits - mg*LSE_g - ms*LSE_s
            t1 = sb.tile([P, Q], F32, tag="t1")
            nc.vector.tensor_scalar(
                out=t1[:], in0=mg2[:], scalar1=lseb[:, 0:1], scalar2=None,
                op0=AL.mult,
            )
            t2 = sb.tile([P, Q], F32, tag="t2")
            nc.vector.tensor_scalar(
                out=t2[:], in0=ms2[:], scalar1=lseb[:, 1:2], scalar2=None,
                op0=AL.mult,
            )
            nc.vector.tensor_tensor(out=t1[:], in0=t1[:], in1=t2[:], op=AL.add)
            out2 = sb.tile([P, Q], F32, tag="out2")
            nc.vector.tensor_tensor(
                out=out2[:], in0=logits2[:], in1=t1[:], op=AL.subtract
            )
            nc.sync.dma_start(out_d[:], out2[:])

            # debug output: total matched-edge count for this core
            nvalid = sb.tile([P, 1], F32, tag="nvalid")
            nc.vector.reduce_sum(nvalid[:], valid[:], axis=AX.X)
            tnv = pst.tile([1, P], F32, tag="trv")
            nc.tensor.transpose(out=tnv[:], in_=nvalid[:], identity=ident[:])
            nvrow = sb.tile([1, P], F32, tag="nvrow")
            nc.vector.tensor_copy(nvrow[:], tnv[:])
            nc.vector.reduce_sum(dbg_t[0:1, 0:1], nvrow[:], axis=AX.X)
            nc.sync.dma_start(dbg_d[:], dbg_t[:])

    nc.compile()
    return nc


def _shuffle_head_vec(vec: np.ndarray) -> np.ndarray:
    """[HSH] -> [P, Q] with head row 512*jb + 4*p + t at [p, 4*jb + t]."""
    return np.ascontiguousarray(
        vec.reshape(JB, P, 4).transpose(1, 0, 2).reshape(P, Q))


def make_in_maps(x, edge_index, edge_type, W_rel, W_root, b_conv,
                 W_global, b_global, W_sense, b_sense):
    x = np.ascontiguousarray(np.asarray(x, dtype=np.float32))
    src_all = np.asarray(edge_index[0], dtype=np.int32)
    tgt_all = np.asarray(edge_index[1], dtype=np.int32)
    typ_all = np.asarray(edge_type, dtype=np.int32)

    waug = np.ascontiguousarray(np.concatenate(
        [np.asarray(W_rel, dtype=np.float32).reshape(R * C, C),
         np.asarray(W_root, dtype=np.float32)], axis=0))
    # b_conv transposed to the h0T layout: [128, 2] with [:, h] = b[128h + p]
    bconv2 = np.ascontiguousarray(
        np.asarray(b_conv, dtype=np.float32).reshape(CH, P).T)

    w_head = np.concatenate(
        [np.asarray(W_global, dtype=np.float32),
         np.asarray(W_sense, dtype=np.float32),
         np.zeros((HEAD_PAD - HEAD, C), dtype=np.float32)], axis=0)
    w_head_T = np.ascontiguousarray(w_head.T)          # [C, HEAD_PAD]
    bh_vec = np.concatenate(
        [np.asarray(b_global, dtype=np.float32),
         np.asarray(b_sense, dtype=np.float32),
         np.zeros(HEAD_PAD - HEAD, dtype=np.float32)])
    jab = np.arange(HEAD_PAD)
    mg_vec = (jab < GLOBALS).astype(np.float32)
    ms_vec = ((jab >= GLOBALS) & (jab < HEAD)).astype(np.float32)
    mgneg_vec = (1.0 - mg_vec) * NEG
    msneg_vec = (1.0 - ms_vec) * NEG

    in_maps = []
    for i in range(NCORES):
        sl = slice(i * ESH, (i + 1) * ESH)
        pad = SH_PAD - ESH
        src_i = np.concatenate([src_all[sl], np.zeros(pad, np.int32)])
        tgt_i = np.concatenate([tgt_all[sl], np.ones(pad, np.int32)])
        typ_i = np.concatenate([typ_all[sl], np.zeros(pad, np.int32)])
        hs = slice(i * HSH, (i + 1) * HSH)
        in_maps.append({
            "x": x,
            "src": np.ascontiguousarray(src_i.reshape(P, FREE)),
            "tgt": np.ascontiguousarray(tgt_i.reshape(P, FREE)),
            "typ": np.ascontiguousarray(typ_i.reshape(P, FREE)),
            "waug": waug,
            "bconv": bconv2,
            "whT": np.ascontiguousarray(w_head_T[:, hs]),
            "bh": _shuffle_head_vec(bh_vec[hs]),
            "mg": _shuffle_head_vec(mg_vec[hs]),
            "ms": _shuffle_head_vec(ms_vec[hs]),
            "mgneg": _shuffle_head_vec(mgneg_vec[hs]),
            "msneg": _shuffle_head_vec(msneg_vec[hs]),
        })
    return in_maps


def postprocess(outs: list) -> tuple:
    flat = np.concatenate([
        np.asarray(o).reshape(P, JB, 4).transpose(1, 0, 2).reshape(-1)
        for o in outs])
    return (np.ascontiguousarray(flat[:GLOBALS], dtype=np.float32),
            np.ascontiguousarray(flat[GLOBALS:HEAD], dtype=np.float32))


_NC_CACHE = None


def kernel(**inputs) -> tuple:
    global _NC_CACHE
    if _NC_CACHE is None:
        _NC_CACHE = build_program()
    nc = _NC_CACHE
    in_maps = make_in_maps(**inputs)
    res = run_bass_kernel_spmd(nc, in_maps, list(range(NCORES)))
    return postprocess([r["out"] for r in res.results])
